# revision 13
# baseline (speedup 1.0000x reference)
"""Trainium2 Bass kernel for nn_AttentionFilter (B,N,D,H = 64,512,1024,2048).

reference:
    energy  = sigmoid(sum(x*x, -1))                      # [B, N]
    hidden  = relu(x @ W1.T + b1)                        # [B, N, H]
    score   = hidden @ W2.T + b2                         # [B, N]
    attn    = sigmoid(score) * energy                    # [B, N]
    filter  = diag_embed(attn)                           # [B, N, N]

Strategy: data-parallel over B across 8 cores (8 batches/core), weights
replicated.  Per core the dominant work is GEMM1 (x @ W1.T, ~17 GFLOP):
computed as hidden^T = W1T-stationary @ x^T with d on partitions,
accumulating 8 d-tiles into PSUM per 128-wide h-tile, 512 tokens per
moving operand.  GEMM operands use float32r (full PE rate at N>=256 vs
4 cycles/row for plain fp32; ~1e-4 rel err).  W1^T streams in ht-major
slabs loaded lazily during batch 0 so the PE starts after ~2.5MB of DMA
instead of 10MB.  ReLU+bias is the ScalarE PSUM->SBUF copy; GEMM2
(score, M=1) runs on the otherwise-idle DVE as per-partition
multiply-accumulate (scalar_tensor_tensor), finished by a ones-matmul
partition reduction; the energy gate is square (ScalarE) + free-dim
partial sums (DVE) + the same ones-matmul reduction.  The diagonal
filter output is built as 4 [128,128] diagonal blocks per batch
(broadcast-matmul of the attn row times an identity mask) and only
those blocks are written -- ExternalOutput DRAM is pre-zeroed by the
runtime on both the native and PJRT paths.
"""

import os

import numpy as np

B, N, D, H = 64, 512, 1024, 2048
NCORES = 8
BPC = B // NCORES  # batches per core
DK = D // 128      # d-tiles (contraction)
HT = H // 128      # h-tiles
CH = N // 128      # 128-row chunks per batch row block

_BUILT = {}
LAST_RESULTS = None  # stashed BassKernelResults for test harness inspection


def _build(mm1="f32r", red="f32r"):
    import dataclasses

    import concourse.bacc as bacc
    import concourse.tile as tile
    from concourse import masks, mybir
    from concourse.tile_rust import add_dep_helper

    nc = bacc.Bacc("TRN2", target_bir_lowering=False, debug=False)

    # float32r ("rounded fp32") runs the PE at full rate (1 cycle/row for
    # N>=256) vs 4 cycles/row for plain fp32.  The BIR verifier requires
    # every producer of an f32r matmul operand to emit f32r, so the whole
    # GEMM-facing chain (DRAM param -> DMA -> SBUF tile) matches.
    DTMAP = {"f32": mybir.dt.float32, "f32r": mybir.dt.float32r,
             "bf16": mybir.dt.bfloat16}
    DT1 = DTMAP[mm1]   # GEMM1 operands: w1 slabs, xtt
    DTR = DTMAP[red]   # partition-reduction matmul operands

    F32 = mybir.dt.float32
    # host pre-tiles x and W1 so every DMA is 128 contiguous rows
    # (16KB / 4KB descriptors instead of 2KB / 512B strided ones)
    xt = nc.declare_dram_parameter("xt", [BPC, 128, DK * N], DT1, isOutput=False)
    w1h = nc.declare_dram_parameter("w1h", [HT, 128, DK * 128], DT1, isOutput=False)
    b1 = nc.declare_dram_parameter("b1", [H], F32, isOutput=False)
    w2 = nc.declare_dram_parameter("w2", [H], F32, isOutput=False)
    b2 = nc.declare_dram_parameter("b2", [1], F32, isOutput=False)
    filt = nc.declare_dram_parameter("filt", [BPC, N, N], F32, isOutput=True)
    attn = nc.declare_dram_parameter("attn", [BPC, N], F32, isOutput=True)

    ACT = mybir.ActivationFunctionType
    ALU = mybir.AluOpType

    with tile.TileContext(nc) as tc:
        with tc.tile_pool(name="singles", bufs=1) as singles, \
             tc.tile_pool(name="work", bufs=1) as work, \
             tc.tile_pool(name="psum", bufs=1, space="PSUM") as psum:

            # ---- small resident constants (weight slabs stream lazily) ----
            b1sb = singles.tile([128, HT], F32, tag="b1sb", name="b1sb")
            nc.sync.dma_start(out=b1sb, in_=b1[:].rearrange("(h p) -> p h", p=128))
            w2sb = singles.tile([128, HT], F32, tag="w2sb", name="w2sb")
            nc.sync.dma_start(out=w2sb, in_=w2[:].rearrange("(h p) -> p h", p=128))
            b2sb = singles.tile([1, 1], F32, tag="b2sb", name="b2sb")
            nc.sync.dma_start(out=b2sb, in_=b2[:].rearrange("(a b) -> a b", a=1))
            ones_row = singles.tile([1, 128], F32, tag="ones_row", name="ones_row")
            nc.vector.memset(ones_row, 1.0)
            ones_row_r = singles.tile([1, 128], DTR, tag="ones_row_r",
                                      name="ones_row_r")
            nc.scalar.copy(out=ones_row_r, in_=ones_row)
            ones_col = singles.tile([128, 1], DTR, tag="ones_col", name="ones_col")
            if DTR == mybir.dt.float32r:
                # memset can't emit f32r; produce it via an ACT copy instead
                ones_f32 = singles.tile([128, 1], F32, tag="ones_f32", name="ones_f32")
                nc.vector.memset(ones_f32, 1.0)
                nc.scalar.copy(out=ones_col, in_=ones_f32)
            else:
                nc.vector.memset(ones_col, 1.0)
            id128 = singles.tile([128, 128], F32, tag="id128", name="id128")
            masks.make_identity(nc, id128)

            # W1^T as 16 ht-major slabs [128 (d%128), DK*128 (dk, h%128)];
            # slab ht's DMA is emitted at its first use (batch 0, h-tile ht)
            # so the first matmul only waits for slab 0 + batch 0's x.
            w1ht = [singles.tile([128, DK * 128], DT1, tag=f"w1h_{ht}",
                                 name=f"w1h_{ht}") for ht in range(HT)]

            # Dummy matmuls on the already-memset ones_row keep the PE busy
            # (and HAM-warm) while the first weight slab + x slab DMA in.
            warm_ps = psum.tile([128, 128], F32, tag="bc", bufs=2, name="warm_ps")
            for _ in range(18):
                nc.tensor.matmul(warm_ps, ones_row, ones_row,
                                 start=True, stop=True)

            b0_group_mm = []  # first matmul instruction of batch-0 group ht
            for b in range(BPC):
                # x^T slab for this batch: partitions = d%128, free = (dk, tok)
                xtt = work.tile([128, DK * N], DT1, tag="xtt", bufs=3, name="xtt")
                if b == 0:
                    # critical startup path: slab 0 + x chunk 0 gate the first
                    # matmul -- load them first, stream the rest behind
                    nc.sync.dma_start(out=w1ht[0], in_=w1h[0])
                    for dk in range(DK):
                        nc.sync.dma_start(out=xtt[:, dk * N:(dk + 1) * N],
                                          in_=xt[b, :, dk * N:(dk + 1) * N])
                else:
                    xdma = nc.sync.dma_start(out=xtt, in_=xt[b])
                    if b == 1:
                        # keep the batch-1 prefetch off the HBM bus during the
                        # batch-0 ramp: start it once batch 0 is well underway
                        add_dep_helper(xdma.ins, b0_group_mm[4],
                                       reason="defer b1 x prefetch past ramp")

                # energy partial: sum over d of x^2, free-dim part on DVE
                sq = work.tile([128, DK * N], F32, tag="sq", bufs=2, name="sq")
                nc.scalar.square(out=sq, in_=xtt.bitcast(F32)
                                 if DT1 == mybir.dt.float32r else xtt)
                sqacc = work.tile([128, N], DTR, tag="sqacc", bufs=2, name="sqacc")
                nc.vector.tensor_add(out=sqacc, in0=sq[:, 0:N], in1=sq[:, N:2 * N])
                for dk in range(2, DK):
                    nc.vector.tensor_add(out=sqacc, in0=sqacc,
                                         in1=sq[:, dk * N:(dk + 1) * N])

                score_acc = work.tile([128, N], DTR, tag="score_acc", bufs=2,
                                      name="score_acc")
                for ht in range(HT):
                    if b == 0 and ht > 0:
                        sdma = nc.sync.dma_start(out=w1ht[ht], in_=w1h[ht])
                        if ht >= 2:
                            # trickle the remaining weight slabs at compute
                            # pace instead of hogging HBM during the ramp
                            add_dep_helper(sdma.ins, b0_group_mm[ht - 2],
                                           reason="pace slab stream")
                    h_ps = psum.tile([128, N], F32, tag="hps", bufs=3, name="h_ps")
                    for dk in range(DK):
                        mm = nc.tensor.matmul(
                            h_ps,
                            w1ht[ht][:, dk * 128:(dk + 1) * 128],
                            xtt[:, dk * N:(dk + 1) * N],
                            start=(dk == 0), stop=(dk == DK - 1),
                        )
                        if b == 0 and dk == 0:
                            b0_group_mm.append(mm.ins)
                    hid = work.tile([128, N], F32, tag="hid", bufs=3, name="hid")
                    nc.scalar.activation(hid, h_ps, ACT.Relu, bias=b1sb[:, ht:ht + 1])
                    # GEMM2 on DVE: score_acc[p,t] += hid[p,t] * w2[128*ht+p]
                    if ht == 0:
                        nc.vector.tensor_scalar_mul(score_acc, hid, w2sb[:, 0:1])
                    else:
                        nc.vector.scalar_tensor_tensor(
                            out=score_acc, in0=hid, scalar=w2sb[:, ht:ht + 1],
                            in1=score_acc, op0=ALU.mult, op1=ALU.add)

                # partition reductions (ones-matmul): score and energy rows
                score_ps = psum.tile([1, N], F32, tag="red", bufs=2, name="score_ps")
                nc.tensor.matmul(score_ps, ones_col, score_acc, start=True, stop=True)
                energy_ps = psum.tile([1, N], F32, tag="red", bufs=2, name="energy_ps")
                nc.tensor.matmul(energy_ps, ones_col, sqacc, start=True, stop=True)

                sig_s = work.tile([1, N], F32, tag="sig_s", bufs=2, name="sig_s")
                nc.scalar.activation(sig_s, score_ps, ACT.Sigmoid, bias=b2sb[0:1, 0:1])
                sig_e = work.tile([1, N], F32, tag="sig_e", bufs=2, name="sig_e")
                nc.scalar.activation(sig_e, energy_ps, ACT.Sigmoid)
                attn_row = work.tile([1, N], F32, tag="attn_row", bufs=2,
                                     name="attn_row")
                nc.vector.tensor_mul(out=attn_row, in0=sig_s, in1=sig_e)
                nc.sync.dma_start(out=attn[b:b + 1, :], in_=attn_row)
                attn_row_r = work.tile([1, N], DTR, tag="attn_row_r", bufs=2,
                                       name="attn_row_r")
                nc.scalar.copy(out=attn_row_r, in_=attn_row)

                # diagonal blocks of the filter matrix: one rank-1 matmul
                # replicates the attn row across all 128 partitions, then DVE
                # masks each 128-column chunk with the identity.
                bc_ps = psum.tile([128, N], F32, tag="bc", bufs=2, name="bc_ps")
                nc.tensor.matmul(bc_ps, ones_row_r, attn_row_r,
                                 start=True, stop=True)
                diag4 = work.tile([128, N], F32, tag="diag4", bufs=2, name="diag4")
                id_b = id128[:, :]
                id_b = dataclasses.replace(
                    id_b, ap=[id_b.ap[0], [0, CH]] + id_b.ap[1:])
                nc.vector.tensor_mul(
                    out=diag4.rearrange("p (c q) -> p c q", c=CH),
                    in0=bc_ps.rearrange("p (c q) -> p c q", c=CH),
                    in1=id_b)
                fd = filt[b]
                fd = dataclasses.replace(
                    fd, ap=[[N, 128], [128 * N + 128, CH], [1, 128]])
                nc.sync.dma_start(out=fd,
                                  in_=diag4.rearrange("p (c q) -> p c q", c=CH))

    nc.compile()
    return nc


MM1 = os.environ.get("AF_MM_DTYPE", "f32r")
RED = os.environ.get("AF_RED_DTYPE", "f32r")


def _get_program():
    key = ("v2", MM1, RED)
    if key not in _BUILT:
        _BUILT[key] = _build(mm1=MM1, red=RED)
    return _BUILT[key]


def kernel(x, W1, b1, W2, b2):
    global LAST_RESULTS
    from concourse.bass_utils import run_bass_kernel_spmd

    x = np.asarray(x, dtype=np.float32)
    W1 = np.asarray(W1, dtype=np.float32)
    b1 = np.asarray(b1, dtype=np.float32).reshape(H)
    W2 = np.asarray(W2, dtype=np.float32).reshape(H)
    b2 = np.asarray(b2, dtype=np.float32).reshape(1)

    nc = _get_program()

    if MM1 == "bf16":
        import ml_dtypes
        np1 = ml_dtypes.bfloat16
    else:
        np1 = np.float32

    # w1h[ht, p, dk*128+q] = W1[ht*128+q, dk*128+p]
    w1h = np.ascontiguousarray(
        W1.reshape(HT, 128, DK, 128).transpose(0, 3, 2, 1)
        .reshape(HT, 128, DK * 128)).astype(np1)
    in_maps = []
    for c in range(NCORES):
        # xt[b, p, dk*N+t] = x[core batch b, tok t, d=dk*128+p]
        xs = x[c * BPC:(c + 1) * BPC].reshape(BPC, N, DK, 128)
        xt_host = np.ascontiguousarray(
            xs.transpose(0, 3, 2, 1).reshape(BPC, 128, DK * N)).astype(np1)
        in_maps.append({
            "xt": xt_host,
            "w1h": w1h,
            "b1": b1,
            "w2": W2,
            "b2": b2,
        })

    res = run_bass_kernel_spmd(nc, in_maps, list(range(NCORES)))
    LAST_RESULTS = res

    filt = np.concatenate([res.results[c]["filt"] for c in range(NCORES)], axis=0)
    attn = np.concatenate([res.results[c]["attn"] for c in range(NCORES)], axis=0)
    return filt, attn


# revision 14
# speedup vs baseline: 1.0424x; 1.0424x over previous
"""Trainium2 Bass kernel for nn_AttentionFilter (B,N,D,H = 64,512,1024,2048).

reference:
    energy  = sigmoid(sum(x*x, -1))                      # [B, N]
    hidden  = relu(x @ W1.T + b1)                        # [B, N, H]
    score   = hidden @ W2.T + b2                         # [B, N]
    attn    = sigmoid(score) * energy                    # [B, N]
    filter  = diag_embed(attn)                           # [B, N, N]

Strategy: data-parallel over B across 8 cores (8 batches/core), weights
replicated.  Per core the dominant work is GEMM1 (x @ W1.T, ~17 GFLOP):
computed as hidden^T = W1T-stationary @ x^T with d on partitions,
accumulating 8 d-tiles into PSUM per 128-wide h-tile, 512 tokens per
moving operand.  GEMM operands use float32r (full PE rate at N>=256 vs
4 cycles/row for plain fp32; ~1e-4 rel err).  W1^T streams in ht-major
slabs loaded lazily during batch 0 so the PE starts after ~2.5MB of DMA
instead of 10MB.  ReLU+bias is the ScalarE PSUM->SBUF copy; GEMM2
(score, M=1) runs on the otherwise-idle DVE as per-partition
multiply-accumulate (scalar_tensor_tensor), finished by a ones-matmul
partition reduction; the energy gate is square (ScalarE) + free-dim
partial sums (DVE) + the same ones-matmul reduction.  The diagonal
filter output is built as 4 [128,128] diagonal blocks per batch
(broadcast-matmul of the attn row times an identity mask) and only
those blocks are written -- ExternalOutput DRAM is pre-zeroed by the
runtime on both the native and PJRT paths.
"""

import os

import numpy as np

B, N, D, H = 64, 512, 1024, 2048
NCORES = 8
BPC = B // NCORES  # batches per core
DK = D // 128      # d-tiles (contraction)
HT = H // 128      # h-tiles
CH = N // 128      # 128-row chunks per batch row block

_BUILT = {}
LAST_RESULTS = None  # stashed BassKernelResults for test harness inspection


def _build(mm1="f32r", red="f32r"):
    import dataclasses

    import concourse.bacc as bacc
    import concourse.tile as tile
    from concourse import masks, mybir

    nc = bacc.Bacc("TRN2", target_bir_lowering=False, debug=False)

    # float32r ("rounded fp32") runs the PE at full rate (1 cycle/row for
    # N>=256) vs 4 cycles/row for plain fp32.  The BIR verifier requires
    # every producer of an f32r matmul operand to emit f32r, so the whole
    # GEMM-facing chain (DRAM param -> DMA -> SBUF tile) matches.
    DTMAP = {"f32": mybir.dt.float32, "f32r": mybir.dt.float32r,
             "bf16": mybir.dt.bfloat16}
    DT1 = DTMAP[mm1]   # GEMM1 operands: w1 slabs, xtt
    DTR = DTMAP[red]   # partition-reduction matmul operands

    F32 = mybir.dt.float32
    # host pre-tiles x and W1 so every DMA is 128 contiguous rows
    # (16KB / 4KB descriptors instead of 2KB / 512B strided ones)
    xt = nc.declare_dram_parameter("xt", [BPC, 128, DK * N], DT1, isOutput=False)
    w1h = nc.declare_dram_parameter("w1h", [HT, 128, DK * 128], DT1, isOutput=False)
    b1 = nc.declare_dram_parameter("b1", [H], F32, isOutput=False)
    w2 = nc.declare_dram_parameter("w2", [H], F32, isOutput=False)
    b2 = nc.declare_dram_parameter("b2", [1], F32, isOutput=False)
    filt = nc.declare_dram_parameter("filt", [BPC, N, N], F32, isOutput=True)
    attn = nc.declare_dram_parameter("attn", [BPC, N], F32, isOutput=True)

    ACT = mybir.ActivationFunctionType
    ALU = mybir.AluOpType

    with tile.TileContext(nc) as tc:
        with tc.tile_pool(name="singles", bufs=1) as singles, \
             tc.tile_pool(name="work", bufs=1) as work, \
             tc.tile_pool(name="psum", bufs=1, space="PSUM") as psum:

            # ---- small resident constants (weight slabs stream lazily) ----
            b1sb = singles.tile([128, HT], F32, tag="b1sb", name="b1sb")
            nc.sync.dma_start(out=b1sb, in_=b1[:].rearrange("(h p) -> p h", p=128))
            w2sb = singles.tile([128, HT], F32, tag="w2sb", name="w2sb")
            nc.sync.dma_start(out=w2sb, in_=w2[:].rearrange("(h p) -> p h", p=128))
            b2sb = singles.tile([1, 1], F32, tag="b2sb", name="b2sb")
            nc.sync.dma_start(out=b2sb, in_=b2[:].rearrange("(a b) -> a b", a=1))
            ones_row = singles.tile([1, 128], F32, tag="ones_row", name="ones_row")
            nc.vector.memset(ones_row, 1.0)
            ones_row_r = singles.tile([1, 128], DTR, tag="ones_row_r",
                                      name="ones_row_r")
            nc.scalar.copy(out=ones_row_r, in_=ones_row)
            ones_col = singles.tile([128, 1], DTR, tag="ones_col", name="ones_col")
            if DTR == mybir.dt.float32r:
                # memset can't emit f32r; produce it via an ACT copy instead
                ones_f32 = singles.tile([128, 1], F32, tag="ones_f32", name="ones_f32")
                nc.vector.memset(ones_f32, 1.0)
                nc.scalar.copy(out=ones_col, in_=ones_f32)
            else:
                nc.vector.memset(ones_col, 1.0)
            id128 = singles.tile([128, 128], F32, tag="id128", name="id128")
            masks.make_identity(nc, id128)

            # W1^T as 16 ht-major slabs [128 (d%128), DK*128 (dk, h%128)];
            # slab ht's DMA is emitted at its first use (batch 0, h-tile ht)
            # so the first matmul only waits for slab 0 + batch 0's x.
            w1ht = [singles.tile([128, DK * 128], DT1, tag=f"w1h_{ht}",
                                 name=f"w1h_{ht}") for ht in range(HT)]

            # Dummy matmuls on the already-memset ones_row keep the PE busy
            # (and HAM-warm) while the first weight slab + x slab DMA in.
            warm_ps = psum.tile([128, 128], F32, tag="bc", bufs=2, name="warm_ps")
            for _ in range(18):
                nc.tensor.matmul(warm_ps, ones_row, ones_row,
                                 start=True, stop=True)

            for b in range(BPC):
                # x^T slab for this batch: partitions = d%128, free = (dk, tok)
                xtt = work.tile([128, DK * N], DT1, tag="xtt", bufs=3, name="xtt")
                if b == 0:
                    # critical startup path: slab 0 + x chunk 0 gate the first
                    # matmul -- load them first, stream the rest behind
                    nc.sync.dma_start(out=w1ht[0], in_=w1h[0])
                    for dk in range(DK):
                        nc.sync.dma_start(out=xtt[:, dk * N:(dk + 1) * N],
                                          in_=xt[b, :, dk * N:(dk + 1) * N])
                else:
                    nc.sync.dma_start(out=xtt, in_=xt[b])

                # energy partial: sum over d of x^2, free-dim part on DVE
                sq = work.tile([128, DK * N], F32, tag="sq", bufs=2, name="sq")
                nc.scalar.square(out=sq, in_=xtt.bitcast(F32)
                                 if DT1 == mybir.dt.float32r else xtt)
                sqacc = work.tile([128, N], DTR, tag="sqacc", bufs=2, name="sqacc")
                nc.vector.tensor_add(out=sqacc, in0=sq[:, 0:N], in1=sq[:, N:2 * N])
                for dk in range(2, DK):
                    nc.vector.tensor_add(out=sqacc, in0=sqacc,
                                         in1=sq[:, dk * N:(dk + 1) * N])

                score_acc = work.tile([128, N], DTR, tag="score_acc", bufs=2,
                                      name="score_acc")
                for ht in range(HT):
                    if b == 0 and ht > 0:
                        nc.sync.dma_start(out=w1ht[ht], in_=w1h[ht])
                    h_ps = psum.tile([128, N], F32, tag="hps", bufs=3, name="h_ps")
                    for dk in range(DK):
                        nc.tensor.matmul(
                            h_ps,
                            w1ht[ht][:, dk * 128:(dk + 1) * 128],
                            xtt[:, dk * N:(dk + 1) * N],
                            start=(dk == 0), stop=(dk == DK - 1),
                        )
                    hid = work.tile([128, N], F32, tag="hid", bufs=3, name="hid")
                    nc.scalar.activation(hid, h_ps, ACT.Relu, bias=b1sb[:, ht:ht + 1])
                    # GEMM2 on DVE: score_acc[p,t] += hid[p,t] * w2[128*ht+p]
                    if ht == 0:
                        nc.vector.tensor_scalar_mul(score_acc, hid, w2sb[:, 0:1])
                    else:
                        nc.vector.scalar_tensor_tensor(
                            out=score_acc, in0=hid, scalar=w2sb[:, ht:ht + 1],
                            in1=score_acc, op0=ALU.mult, op1=ALU.add)

                # partition reductions (ones-matmul): score and energy rows
                score_ps = psum.tile([1, N], F32, tag="red", bufs=2, name="score_ps")
                nc.tensor.matmul(score_ps, ones_col, score_acc, start=True, stop=True)
                energy_ps = psum.tile([1, N], F32, tag="red", bufs=2, name="energy_ps")
                nc.tensor.matmul(energy_ps, ones_col, sqacc, start=True, stop=True)

                sig_s = work.tile([1, N], F32, tag="sig_s", bufs=2, name="sig_s")
                nc.scalar.activation(sig_s, score_ps, ACT.Sigmoid, bias=b2sb[0:1, 0:1])
                sig_e = work.tile([1, N], F32, tag="sig_e", bufs=2, name="sig_e")
                nc.scalar.activation(sig_e, energy_ps, ACT.Sigmoid)
                attn_row = work.tile([1, N], F32, tag="attn_row", bufs=2,
                                     name="attn_row")
                nc.vector.tensor_mul(out=attn_row, in0=sig_s, in1=sig_e)
                nc.sync.dma_start(out=attn[b:b + 1, :], in_=attn_row)
                attn_row_r = work.tile([1, N], DTR, tag="attn_row_r", bufs=2,
                                       name="attn_row_r")
                nc.scalar.copy(out=attn_row_r, in_=attn_row)

                # diagonal blocks of the filter matrix: one rank-1 matmul
                # replicates the attn row across all 128 partitions, then DVE
                # masks each 128-column chunk with the identity.
                bc_ps = psum.tile([128, N], F32, tag="bc", bufs=2, name="bc_ps")
                nc.tensor.matmul(bc_ps, ones_row_r, attn_row_r,
                                 start=True, stop=True)
                diag4 = work.tile([128, N], F32, tag="diag4", bufs=2, name="diag4")
                id_b = id128[:, :]
                id_b = dataclasses.replace(
                    id_b, ap=[id_b.ap[0], [0, CH]] + id_b.ap[1:])
                nc.vector.tensor_mul(
                    out=diag4.rearrange("p (c q) -> p c q", c=CH),
                    in0=bc_ps.rearrange("p (c q) -> p c q", c=CH),
                    in1=id_b)
                fd = filt[b]
                fd = dataclasses.replace(
                    fd, ap=[[N, 128], [128 * N + 128, CH], [1, 128]])
                nc.sync.dma_start(out=fd,
                                  in_=diag4.rearrange("p (c q) -> p c q", c=CH))

    nc.compile()
    return nc


MM1 = os.environ.get("AF_MM_DTYPE", "f32r")
RED = os.environ.get("AF_RED_DTYPE", "f32r")


def _get_program():
    key = ("v2", MM1, RED)
    if key not in _BUILT:
        _BUILT[key] = _build(mm1=MM1, red=RED)
    return _BUILT[key]


def kernel(x, W1, b1, W2, b2):
    global LAST_RESULTS
    from concourse.bass_utils import run_bass_kernel_spmd

    x = np.asarray(x, dtype=np.float32)
    W1 = np.asarray(W1, dtype=np.float32)
    b1 = np.asarray(b1, dtype=np.float32).reshape(H)
    W2 = np.asarray(W2, dtype=np.float32).reshape(H)
    b2 = np.asarray(b2, dtype=np.float32).reshape(1)

    nc = _get_program()

    if MM1 == "bf16":
        import ml_dtypes
        np1 = ml_dtypes.bfloat16
    else:
        np1 = np.float32

    # w1h[ht, p, dk*128+q] = W1[ht*128+q, dk*128+p]
    w1h = np.ascontiguousarray(
        W1.reshape(HT, 128, DK, 128).transpose(0, 3, 2, 1)
        .reshape(HT, 128, DK * 128)).astype(np1)
    in_maps = []
    for c in range(NCORES):
        # xt[b, p, dk*N+t] = x[core batch b, tok t, d=dk*128+p]
        xs = x[c * BPC:(c + 1) * BPC].reshape(BPC, N, DK, 128)
        xt_host = np.ascontiguousarray(
            xs.transpose(0, 3, 2, 1).reshape(BPC, 128, DK * N)).astype(np1)
        in_maps.append({
            "xt": xt_host,
            "w1h": w1h,
            "b1": b1,
            "w2": W2,
            "b2": b2,
        })

    res = run_bass_kernel_spmd(nc, in_maps, list(range(NCORES)))
    LAST_RESULTS = res

    filt = np.concatenate([res.results[c]["filt"] for c in range(NCORES)], axis=0)
    attn = np.concatenate([res.results[c]["attn"] for c in range(NCORES)], axis=0)
    return filt, attn


# revision 15
# speedup vs baseline: 1.0604x; 1.0173x over previous
"""Trainium2 Bass kernel for nn_AttentionFilter (B,N,D,H = 64,512,1024,2048).

reference:
    energy  = sigmoid(sum(x*x, -1))                      # [B, N]
    hidden  = relu(x @ W1.T + b1)                        # [B, N, H]
    score   = hidden @ W2.T + b2                         # [B, N]
    attn    = sigmoid(score) * energy                    # [B, N]
    filter  = diag_embed(attn)                           # [B, N, N]

Strategy: data-parallel over B across 8 cores (8 batches/core), weights
replicated.  Per core the dominant work is GEMM1 (x @ W1.T, ~17 GFLOP):
computed as hidden^T = W1T-stationary @ x^T with d on partitions,
accumulating 8 d-tiles into PSUM per 128-wide h-tile, 512 tokens per
moving operand.  GEMM operands use float32r (full PE rate at N>=256 vs
4 cycles/row for plain fp32; ~1e-4 rel err).  W1^T streams in ht-major
slabs loaded lazily during batch 0 so the PE starts after ~2.5MB of DMA
instead of 10MB.  ReLU+bias is the ScalarE PSUM->SBUF copy; GEMM2
(score, M=1) runs on the otherwise-idle DVE as per-partition
multiply-accumulate (scalar_tensor_tensor), finished by a ones-matmul
partition reduction; the energy gate is square (ScalarE) + free-dim
partial sums (DVE) + the same ones-matmul reduction.  The diagonal
filter output is built as 4 [128,128] diagonal blocks per batch
(broadcast-matmul of the attn row times an identity mask) and only
those blocks are written -- ExternalOutput DRAM is pre-zeroed by the
runtime on both the native and PJRT paths.
"""

import os

import numpy as np

B, N, D, H = 64, 512, 1024, 2048
NCORES = 8
BPC = B // NCORES  # batches per core
DK = D // 128      # d-tiles (contraction)
HT = H // 128      # h-tiles
CH = N // 128      # 128-row chunks per batch row block

_BUILT = {}
LAST_RESULTS = None  # stashed BassKernelResults for test harness inspection


def _build(mm1="f32r", red="f32r"):
    import dataclasses

    import concourse.bacc as bacc
    import concourse.tile as tile
    from concourse import masks, mybir

    nc = bacc.Bacc("TRN2", target_bir_lowering=False, debug=False)

    # float32r ("rounded fp32") runs the PE at full rate (1 cycle/row for
    # N>=256) vs 4 cycles/row for plain fp32.  The BIR verifier requires
    # every producer of an f32r matmul operand to emit f32r, so the whole
    # GEMM-facing chain (DRAM param -> DMA -> SBUF tile) matches.
    DTMAP = {"f32": mybir.dt.float32, "f32r": mybir.dt.float32r,
             "bf16": mybir.dt.bfloat16}
    DT1 = DTMAP[mm1]   # GEMM1 operands: w1 slabs, xtt
    DTR = DTMAP[red]   # partition-reduction matmul operands

    F32 = mybir.dt.float32
    # host pre-tiles x and W1 so every DMA is 128 contiguous rows
    # (16KB / 4KB descriptors instead of 2KB / 512B strided ones)
    xt = nc.declare_dram_parameter("xt", [BPC, 128, DK * N], DT1, isOutput=False)
    w1h = nc.declare_dram_parameter("w1h", [HT, 128, DK * 128], DT1, isOutput=False)
    b1 = nc.declare_dram_parameter("b1", [H], F32, isOutput=False)
    w2 = nc.declare_dram_parameter("w2", [H], F32, isOutput=False)
    b2 = nc.declare_dram_parameter("b2", [1], F32, isOutput=False)
    filt = nc.declare_dram_parameter("filt", [BPC, N, N], F32, isOutput=True)
    attn = nc.declare_dram_parameter("attn", [BPC, N], F32, isOutput=True)

    ACT = mybir.ActivationFunctionType
    ALU = mybir.AluOpType

    with tile.TileContext(nc) as tc:
        with tc.tile_pool(name="singles", bufs=1) as singles, \
             tc.tile_pool(name="work", bufs=1) as work, \
             tc.tile_pool(name="psum", bufs=1, space="PSUM") as psum:

            # ---- small resident constants (weight slabs stream lazily) ----
            b1sb = singles.tile([128, HT], F32, tag="b1sb", name="b1sb")
            nc.sync.dma_start(out=b1sb, in_=b1[:].rearrange("(h p) -> p h", p=128))
            w2sb = singles.tile([128, HT], F32, tag="w2sb", name="w2sb")
            nc.sync.dma_start(out=w2sb, in_=w2[:].rearrange("(h p) -> p h", p=128))
            b2sb = singles.tile([1, 1], F32, tag="b2sb", name="b2sb")
            nc.sync.dma_start(out=b2sb, in_=b2[:].rearrange("(a b) -> a b", a=1))
            ones_row = singles.tile([1, 128], F32, tag="ones_row", name="ones_row")
            nc.vector.memset(ones_row, 1.0)
            ones_row_r = singles.tile([1, 128], DTR, tag="ones_row_r",
                                      name="ones_row_r")
            nc.scalar.copy(out=ones_row_r, in_=ones_row)
            ones_col = singles.tile([128, 1], DTR, tag="ones_col", name="ones_col")
            if DTR == mybir.dt.float32r:
                # memset can't emit f32r; produce it via an ACT copy instead
                ones_f32 = singles.tile([128, 1], F32, tag="ones_f32", name="ones_f32")
                nc.vector.memset(ones_f32, 1.0)
                nc.scalar.copy(out=ones_col, in_=ones_f32)
            else:
                nc.vector.memset(ones_col, 1.0)
            id128 = singles.tile([128, 128], F32, tag="id128", name="id128")
            masks.make_identity(nc, id128)

            # W1^T as 16 ht-major slabs [128 (d%128), DK*128 (dk, h%128)];
            # slab ht's DMA is emitted at its first use (batch 0, h-tile ht)
            # so the first matmul only waits for slab 0 + batch 0's x.
            w1ht = [singles.tile([128, DK * 128], DT1, tag=f"w1h_{ht}",
                                 name=f"w1h_{ht}") for ht in range(HT)]

            # Dummy matmuls on the already-memset ones_row keep the PE busy
            # (and HAM-warm) while the first weight slab + x slab DMA in.
            warm_ps = psum.tile([128, 128], F32, tag="bc", bufs=2, name="warm_ps")
            for _ in range(18):
                nc.tensor.matmul(warm_ps, ones_row, ones_row,
                                 start=True, stop=True)

            for b in range(BPC):
                # x^T slab for this batch: partitions = d%128, free = (dk, tok)
                xtt = work.tile([128, DK * N], DT1, tag="xtt", bufs=4, name="xtt")
                if b == 0:
                    # critical startup path: slab 0 + x chunk 0 gate the first
                    # matmul -- load them first, stream the rest behind
                    nc.sync.dma_start(out=w1ht[0], in_=w1h[0])
                    for dk in range(DK):
                        nc.sync.dma_start(out=xtt[:, dk * N:(dk + 1) * N],
                                          in_=xt[b, :, dk * N:(dk + 1) * N])
                else:
                    nc.sync.dma_start(out=xtt, in_=xt[b])

                # energy partial: sum over d of x^2, free-dim part on DVE
                sq = work.tile([128, DK * N], F32, tag="sq", bufs=2, name="sq")
                nc.scalar.square(out=sq, in_=xtt.bitcast(F32)
                                 if DT1 == mybir.dt.float32r else xtt)
                sqacc = work.tile([128, N], DTR, tag="sqacc", bufs=2, name="sqacc")
                nc.vector.tensor_add(out=sqacc, in0=sq[:, 0:N], in1=sq[:, N:2 * N])
                for dk in range(2, DK):
                    nc.vector.tensor_add(out=sqacc, in0=sqacc,
                                         in1=sq[:, dk * N:(dk + 1) * N])

                score_acc = work.tile([128, N], DTR, tag="score_acc", bufs=2,
                                      name="score_acc")
                for ht in range(HT):
                    if b == 0 and ht > 0:
                        nc.sync.dma_start(out=w1ht[ht], in_=w1h[ht])
                    h_ps = psum.tile([128, N], F32, tag="hps", bufs=4, name="h_ps")
                    for dk in range(DK):
                        nc.tensor.matmul(
                            h_ps,
                            w1ht[ht][:, dk * 128:(dk + 1) * 128],
                            xtt[:, dk * N:(dk + 1) * N],
                            start=(dk == 0), stop=(dk == DK - 1),
                        )
                    hid = work.tile([128, N], F32, tag="hid", bufs=3, name="hid")
                    nc.scalar.activation(hid, h_ps, ACT.Relu, bias=b1sb[:, ht:ht + 1])
                    # GEMM2 on DVE: score_acc[p,t] += hid[p,t] * w2[128*ht+p]
                    if ht == 0:
                        nc.vector.tensor_scalar_mul(score_acc, hid, w2sb[:, 0:1])
                    else:
                        nc.vector.scalar_tensor_tensor(
                            out=score_acc, in0=hid, scalar=w2sb[:, ht:ht + 1],
                            in1=score_acc, op0=ALU.mult, op1=ALU.add)

                # partition reductions (ones-matmul): score and energy rows
                score_ps = psum.tile([1, N], F32, tag="red", bufs=2, name="score_ps")
                nc.tensor.matmul(score_ps, ones_col, score_acc, start=True, stop=True)
                energy_ps = psum.tile([1, N], F32, tag="red", bufs=2, name="energy_ps")
                nc.tensor.matmul(energy_ps, ones_col, sqacc, start=True, stop=True)

                sig_s = work.tile([1, N], F32, tag="sig_s", bufs=2, name="sig_s")
                nc.scalar.activation(sig_s, score_ps, ACT.Sigmoid, bias=b2sb[0:1, 0:1])
                sig_e = work.tile([1, N], F32, tag="sig_e", bufs=2, name="sig_e")
                nc.scalar.activation(sig_e, energy_ps, ACT.Sigmoid)
                attn_row = work.tile([1, N], F32, tag="attn_row", bufs=2,
                                     name="attn_row")
                nc.vector.tensor_mul(out=attn_row, in0=sig_s, in1=sig_e)
                nc.sync.dma_start(out=attn[b:b + 1, :], in_=attn_row)
                attn_row_r = work.tile([1, N], DTR, tag="attn_row_r", bufs=2,
                                       name="attn_row_r")
                nc.scalar.copy(out=attn_row_r, in_=attn_row)

                # diagonal blocks of the filter matrix: one rank-1 matmul
                # replicates the attn row across all 128 partitions, then DVE
                # masks each 128-column chunk with the identity.
                bc_ps = psum.tile([128, N], F32, tag="bc", bufs=2, name="bc_ps")
                nc.tensor.matmul(bc_ps, ones_row_r, attn_row_r,
                                 start=True, stop=True)
                diag4 = work.tile([128, N], F32, tag="diag4", bufs=2, name="diag4")
                id_b = id128[:, :]
                id_b = dataclasses.replace(
                    id_b, ap=[id_b.ap[0], [0, CH]] + id_b.ap[1:])
                nc.vector.tensor_mul(
                    out=diag4.rearrange("p (c q) -> p c q", c=CH),
                    in0=bc_ps.rearrange("p (c q) -> p c q", c=CH),
                    in1=id_b)
                fd = filt[b]
                fd = dataclasses.replace(
                    fd, ap=[[N, 128], [128 * N + 128, CH], [1, 128]])
                nc.sync.dma_start(out=fd,
                                  in_=diag4.rearrange("p (c q) -> p c q", c=CH))

    nc.compile()
    return nc


MM1 = os.environ.get("AF_MM_DTYPE", "f32r")
RED = os.environ.get("AF_RED_DTYPE", "f32r")


def _get_program():
    key = ("v2", MM1, RED)
    if key not in _BUILT:
        _BUILT[key] = _build(mm1=MM1, red=RED)
    return _BUILT[key]


def kernel(x, W1, b1, W2, b2):
    global LAST_RESULTS
    from concourse.bass_utils import run_bass_kernel_spmd

    x = np.asarray(x, dtype=np.float32)
    W1 = np.asarray(W1, dtype=np.float32)
    b1 = np.asarray(b1, dtype=np.float32).reshape(H)
    W2 = np.asarray(W2, dtype=np.float32).reshape(H)
    b2 = np.asarray(b2, dtype=np.float32).reshape(1)

    nc = _get_program()

    if MM1 == "bf16":
        import ml_dtypes
        np1 = ml_dtypes.bfloat16
    else:
        np1 = np.float32

    # w1h[ht, p, dk*128+q] = W1[ht*128+q, dk*128+p]
    w1h = np.ascontiguousarray(
        W1.reshape(HT, 128, DK, 128).transpose(0, 3, 2, 1)
        .reshape(HT, 128, DK * 128)).astype(np1)
    in_maps = []
    for c in range(NCORES):
        # xt[b, p, dk*N+t] = x[core batch b, tok t, d=dk*128+p]
        xs = x[c * BPC:(c + 1) * BPC].reshape(BPC, N, DK, 128)
        xt_host = np.ascontiguousarray(
            xs.transpose(0, 3, 2, 1).reshape(BPC, 128, DK * N)).astype(np1)
        in_maps.append({
            "xt": xt_host,
            "w1h": w1h,
            "b1": b1,
            "w2": W2,
            "b2": b2,
        })

    res = run_bass_kernel_spmd(nc, in_maps, list(range(NCORES)))
    LAST_RESULTS = res

    filt = np.concatenate([res.results[c]["filt"] for c in range(NCORES)], axis=0)
    attn = np.concatenate([res.results[c]["attn"] for c in range(NCORES)], axis=0)
    return filt, attn


# revision 16
# speedup vs baseline: 1.0615x; 1.0011x over previous
"""Trainium2 Bass kernel for nn_AttentionFilter (B,N,D,H = 64,512,1024,2048).

reference:
    energy  = sigmoid(sum(x*x, -1))                      # [B, N]
    hidden  = relu(x @ W1.T + b1)                        # [B, N, H]
    score   = hidden @ W2.T + b2                         # [B, N]
    attn    = sigmoid(score) * energy                    # [B, N]
    filter  = diag_embed(attn)                           # [B, N, N]

Strategy: data-parallel over B across 8 cores (8 batches/core), weights
replicated.  Per core the dominant work is GEMM1 (x @ W1.T, ~17 GFLOP):
computed as hidden^T = W1T-stationary @ x^T with d on partitions,
accumulating 8 d-tiles into PSUM per 128-wide h-tile, 512 tokens per
moving operand.  GEMM operands use float32r (full PE rate at N>=256 vs
4 cycles/row for plain fp32; ~1e-4 rel err).  W1^T streams in ht-major
slabs loaded lazily during batch 0 so the PE starts after ~2.5MB of DMA
instead of 10MB.  ReLU+bias is the ScalarE PSUM->SBUF copy; GEMM2
(score, M=1) runs on the otherwise-idle DVE as per-partition
multiply-accumulate (scalar_tensor_tensor), finished by a ones-matmul
partition reduction; the energy gate is square (ScalarE) + free-dim
partial sums (DVE) + the same ones-matmul reduction.  The diagonal
filter output is built as 4 [128,128] diagonal blocks per batch
(broadcast-matmul of the attn row times an identity mask) and only
those blocks are written -- ExternalOutput DRAM is pre-zeroed by the
runtime on both the native and PJRT paths.
"""

import os

import numpy as np

B, N, D, H = 64, 512, 1024, 2048
NCORES = 8
BPC = B // NCORES  # batches per core
DK = D // 128      # d-tiles (contraction)
HT = H // 128      # h-tiles
CH = N // 128      # 128-row chunks per batch row block

_BUILT = {}
LAST_RESULTS = None  # stashed BassKernelResults for test harness inspection


def _build(mm1="f32r", red="f32r"):
    import dataclasses

    import concourse.bacc as bacc
    import concourse.tile as tile
    from concourse import masks, mybir

    nc = bacc.Bacc("TRN2", target_bir_lowering=False, debug=False)

    # float32r ("rounded fp32") runs the PE at full rate (1 cycle/row for
    # N>=256) vs 4 cycles/row for plain fp32.  The BIR verifier requires
    # every producer of an f32r matmul operand to emit f32r, so the whole
    # GEMM-facing chain (DRAM param -> DMA -> SBUF tile) matches.
    DTMAP = {"f32": mybir.dt.float32, "f32r": mybir.dt.float32r,
             "bf16": mybir.dt.bfloat16}
    DT1 = DTMAP[mm1]   # GEMM1 operands: w1 slabs, xtt
    DTR = DTMAP[red]   # partition-reduction matmul operands

    F32 = mybir.dt.float32
    # host pre-tiles x and W1 so every DMA is 128 contiguous rows
    # (16KB / 4KB descriptors instead of 2KB / 512B strided ones)
    xt = nc.declare_dram_parameter("xt", [BPC, 128, DK * N], DT1, isOutput=False)
    w1h = nc.declare_dram_parameter("w1h", [HT, 128, DK * 128], DT1, isOutput=False)
    b1 = nc.declare_dram_parameter("b1", [H], F32, isOutput=False)
    w2 = nc.declare_dram_parameter("w2", [H], F32, isOutput=False)
    b2 = nc.declare_dram_parameter("b2", [1], F32, isOutput=False)
    filt = nc.declare_dram_parameter("filt", [BPC, N, N], F32, isOutput=True)
    attn = nc.declare_dram_parameter("attn", [BPC, N], F32, isOutput=True)

    ACT = mybir.ActivationFunctionType
    ALU = mybir.AluOpType

    with tile.TileContext(nc) as tc:
        with tc.tile_pool(name="singles", bufs=1) as singles, \
             tc.tile_pool(name="work", bufs=1) as work, \
             tc.tile_pool(name="psum", bufs=1, space="PSUM") as psum:

            # ---- small resident constants (weight slabs stream lazily) ----
            b1sb = singles.tile([128, HT], F32, tag="b1sb", name="b1sb")
            nc.sync.dma_start(out=b1sb, in_=b1[:].rearrange("(h p) -> p h", p=128))
            w2sb = singles.tile([128, HT], F32, tag="w2sb", name="w2sb")
            nc.sync.dma_start(out=w2sb, in_=w2[:].rearrange("(h p) -> p h", p=128))
            b2sb = singles.tile([1, 1], F32, tag="b2sb", name="b2sb")
            nc.sync.dma_start(out=b2sb, in_=b2[:].rearrange("(a b) -> a b", a=1))
            ones_row = singles.tile([1, 128], F32, tag="ones_row", name="ones_row")
            nc.vector.memset(ones_row, 1.0)
            ones_row_r = singles.tile([1, 128], DTR, tag="ones_row_r",
                                      name="ones_row_r")
            nc.scalar.copy(out=ones_row_r, in_=ones_row)
            ones_col = singles.tile([128, 1], DTR, tag="ones_col", name="ones_col")
            if DTR == mybir.dt.float32r:
                # memset can't emit f32r; produce it via an ACT copy instead
                ones_f32 = singles.tile([128, 1], F32, tag="ones_f32", name="ones_f32")
                nc.vector.memset(ones_f32, 1.0)
                nc.scalar.copy(out=ones_col, in_=ones_f32)
            else:
                nc.vector.memset(ones_col, 1.0)
            id128 = singles.tile([128, 128], F32, tag="id128", name="id128")
            masks.make_identity(nc, id128)

            # W1^T as 16 ht-major slabs [128 (d%128), DK*128 (dk, h%128)];
            # slab ht's DMA is emitted at its first use (batch 0, h-tile ht)
            # so the first matmul only waits for slab 0 + batch 0's x.
            w1ht = [singles.tile([128, DK * 128], DT1, tag=f"w1h_{ht}",
                                 name=f"w1h_{ht}") for ht in range(HT)]

            # Dummy matmuls on the already-memset ones_row keep the PE busy
            # (and HAM-warm) while the first weight slab + x slab DMA in.
            warm_ps = psum.tile([128, 128], F32, tag="bc", bufs=2, name="warm_ps")
            for _ in range(18):
                nc.tensor.matmul(warm_ps, ones_row, ones_row,
                                 start=True, stop=True)

            pending_finale = None
            for b in range(BPC):
                # x^T slab for this batch: partitions = d%128, free = (dk, tok)
                xtt = work.tile([128, DK * N], DT1, tag="xtt", bufs=4, name="xtt")
                if b == 0:
                    # critical startup path: slab 0 + x chunk 0 gate the first
                    # matmul -- load them first, stream the rest behind
                    nc.sync.dma_start(out=w1ht[0], in_=w1h[0])
                    for dk in range(DK):
                        nc.sync.dma_start(out=xtt[:, dk * N:(dk + 1) * N],
                                          in_=xt[b, :, dk * N:(dk + 1) * N])
                else:
                    nc.sync.dma_start(out=xtt, in_=xt[b])

                # energy partial: sum over d of x^2, free-dim part on DVE
                sq = work.tile([128, DK * N], F32, tag="sq", bufs=2, name="sq")
                nc.scalar.square(out=sq, in_=xtt.bitcast(F32)
                                 if DT1 == mybir.dt.float32r else xtt)
                sqacc = work.tile([128, N], DTR, tag="sqacc", bufs=2, name="sqacc")
                nc.vector.tensor_add(out=sqacc, in0=sq[:, 0:N], in1=sq[:, N:2 * N])
                for dk in range(2, DK):
                    nc.vector.tensor_add(out=sqacc, in0=sqacc,
                                         in1=sq[:, dk * N:(dk + 1) * N])

                score_acc = work.tile([128, N], DTR, tag="score_acc", bufs=2,
                                      name="score_acc")
                for ht in range(HT):
                    if b == 0 and ht > 0:
                        nc.sync.dma_start(out=w1ht[ht], in_=w1h[ht])
                    h_ps = psum.tile([128, N], F32, tag="hps", bufs=4, name="h_ps")
                    for dk in range(DK):
                        nc.tensor.matmul(
                            h_ps,
                            w1ht[ht][:, dk * 128:(dk + 1) * 128],
                            xtt[:, dk * N:(dk + 1) * N],
                            start=(dk == 0), stop=(dk == DK - 1),
                        )
                    hid = work.tile([128, N], F32, tag="hid", bufs=3, name="hid")
                    nc.scalar.activation(hid, h_ps, ACT.Relu, bias=b1sb[:, ht:ht + 1])
                    # GEMM2 on DVE: score_acc[p,t] += hid[p,t] * w2[128*ht+p]
                    if ht == 0:
                        nc.vector.tensor_scalar_mul(score_acc, hid, w2sb[:, 0:1])
                    else:
                        nc.vector.scalar_tensor_tensor(
                            out=score_acc, in0=hid, scalar=w2sb[:, ht:ht + 1],
                            in1=score_acc, op0=ALU.mult, op1=ALU.add)
                    if ht == 2 and pending_finale is not None:
                        # flush the previous batch's finale now: its DVE score
                        # chain finished long ago, so the reduce matmuls slot
                        # into the PE stream without stalling it
                        pending_finale()
                        pending_finale = None

                def finale(b=b, score_acc=score_acc, sqacc=sqacc):
                    # partition reductions (ones-matmul): score + energy rows
                    score_ps = psum.tile([1, N], F32, tag="red", bufs=2,
                                         name="score_ps")
                    nc.tensor.matmul(score_ps, ones_col, score_acc,
                                     start=True, stop=True)
                    energy_ps = psum.tile([1, N], F32, tag="red", bufs=2,
                                          name="energy_ps")
                    nc.tensor.matmul(energy_ps, ones_col, sqacc,
                                     start=True, stop=True)

                    sig_s = work.tile([1, N], F32, tag="sig_s", bufs=2,
                                      name="sig_s")
                    nc.scalar.activation(sig_s, score_ps, ACT.Sigmoid,
                                         bias=b2sb[0:1, 0:1])
                    sig_e = work.tile([1, N], F32, tag="sig_e", bufs=2,
                                      name="sig_e")
                    nc.scalar.activation(sig_e, energy_ps, ACT.Sigmoid)
                    attn_row = work.tile([1, N], F32, tag="attn_row", bufs=2,
                                         name="attn_row")
                    nc.vector.tensor_mul(out=attn_row, in0=sig_s, in1=sig_e)
                    nc.sync.dma_start(out=attn[b:b + 1, :], in_=attn_row)
                    attn_row_r = work.tile([1, N], DTR, tag="attn_row_r",
                                           bufs=2, name="attn_row_r")
                    nc.scalar.copy(out=attn_row_r, in_=attn_row)

                    # diagonal blocks: one rank-1 matmul replicates the attn
                    # row across partitions, DVE masks with the identity
                    bc_ps = psum.tile([128, N], F32, tag="bc", bufs=2,
                                      name="bc_ps")
                    nc.tensor.matmul(bc_ps, ones_row_r, attn_row_r,
                                     start=True, stop=True)
                    diag4 = work.tile([128, N], F32, tag="diag4", bufs=2,
                                      name="diag4")
                    id_b = id128[:, :]
                    id_b = dataclasses.replace(
                        id_b, ap=[id_b.ap[0], [0, CH]] + id_b.ap[1:])
                    nc.vector.tensor_mul(
                        out=diag4.rearrange("p (c q) -> p c q", c=CH),
                        in0=bc_ps.rearrange("p (c q) -> p c q", c=CH),
                        in1=id_b)
                    fd = filt[b]
                    fd = dataclasses.replace(
                        fd, ap=[[N, 128], [128 * N + 128, CH], [1, 128]])
                    nc.sync.dma_start(
                        out=fd, in_=diag4.rearrange("p (c q) -> p c q", c=CH))

                pending_finale = finale

            pending_finale()

    nc.compile()
    return nc


MM1 = os.environ.get("AF_MM_DTYPE", "f32r")
RED = os.environ.get("AF_RED_DTYPE", "f32r")


def _get_program():
    key = ("v2", MM1, RED)
    if key not in _BUILT:
        _BUILT[key] = _build(mm1=MM1, red=RED)
    return _BUILT[key]


def kernel(x, W1, b1, W2, b2):
    global LAST_RESULTS
    from concourse.bass_utils import run_bass_kernel_spmd

    x = np.asarray(x, dtype=np.float32)
    W1 = np.asarray(W1, dtype=np.float32)
    b1 = np.asarray(b1, dtype=np.float32).reshape(H)
    W2 = np.asarray(W2, dtype=np.float32).reshape(H)
    b2 = np.asarray(b2, dtype=np.float32).reshape(1)

    nc = _get_program()

    if MM1 == "bf16":
        import ml_dtypes
        np1 = ml_dtypes.bfloat16
    else:
        np1 = np.float32

    # w1h[ht, p, dk*128+q] = W1[ht*128+q, dk*128+p]
    w1h = np.ascontiguousarray(
        W1.reshape(HT, 128, DK, 128).transpose(0, 3, 2, 1)
        .reshape(HT, 128, DK * 128)).astype(np1)
    in_maps = []
    for c in range(NCORES):
        # xt[b, p, dk*N+t] = x[core batch b, tok t, d=dk*128+p]
        xs = x[c * BPC:(c + 1) * BPC].reshape(BPC, N, DK, 128)
        xt_host = np.ascontiguousarray(
            xs.transpose(0, 3, 2, 1).reshape(BPC, 128, DK * N)).astype(np1)
        in_maps.append({
            "xt": xt_host,
            "w1h": w1h,
            "b1": b1,
            "w2": W2,
            "b2": b2,
        })

    res = run_bass_kernel_spmd(nc, in_maps, list(range(NCORES)))
    LAST_RESULTS = res

    filt = np.concatenate([res.results[c]["filt"] for c in range(NCORES)], axis=0)
    attn = np.concatenate([res.results[c]["attn"] for c in range(NCORES)], axis=0)
    return filt, attn
